# revision 2
# baseline (speedup 1.0000x reference)
"""Trainium2 Bass kernel for nn_Attention_38276748542551.

Llama-style GQA attention block (DIM=4096, 32 q-heads, 8 kv-heads, hd=128,
b=2, s=2048, start_pos=0), tensor-parallel over heads across 8 NeuronCores:
each core owns 4 q-heads / 1 kv-head (wq/wk/wv output-dim shard, wo
input-dim shard) and computes a full [b*s, 4096] partial of the wo output;
the all-reduce is done on the host after gathering the 8 partials.

All matmuls run in float32r (fp32 with 11-bit mantissa; measured 1.3e-4
rel-err over K=4096 — identical to the PE's plain-fp32 mode, 4x faster).

Device dataflow per core:
  phase 1 (per 128-token tile): xT tile (host pre-transposed) -> QKV
  projections (contraction over DIM on partitions) -> rope (deinterleaved
  even/odd layout baked into the weight sharding) -> PE-transpose Q,K to
  feature-major; Q spilled to a DRAM scratch, K^T and V stay in SBUF.
  phase 2 (per batch, per pair of 128-token query tiles): causal scores
  in 512-wide chunks -> masked -> exp (ACT, row sums via accum_out) ->
  1/l scale -> PE-transpose prob tiles -> PV accumulation (N=256) ->
  wo projection per token tile -> DMA partial out.
"""
import sys
import numpy as np

sys.path.insert(0, "/opt/trn_rl_repo")

import concourse.bass as bass  # noqa: E402
import concourse.tile as tile  # noqa: E402
from concourse import bacc, mybir  # noqa: E402
from concourse import bass_utils  # noqa: E402

F32 = mybir.dt.float32
F32R = mybir.dt.float32r
AF = mybir.ActivationFunctionType

DIM = 4096
NK = DIM // 128          # contraction k-tiles
HD = 128                 # head dim
NH_LOC = 4               # q heads per core
QDIM = NH_LOC * HD       # 512
KVDIM = 2 * HD           # K and V projected together, 256
N_CORES = 8
SOFTMAX_SCALE = 1.0 / np.sqrt(HD)


def build_nc(B=2, S=2048):
    """Build the per-core Bass program (identical across cores; data differs)."""
    NT = B * S // 128            # token tiles total
    TPB = S // 128               # token tiles per batch
    NG = TPB // 2                # query-tile pairs per batch

    nc = bacc.Bacc("TRN2", target_bir_lowering=False, debug=False,
                   enable_asserts=False, num_devices=N_CORES)

    x_t = nc.dram_tensor("x_t", [NT, NK, 128, 128], F32, kind="ExternalInput").ap()
    wq_t = nc.dram_tensor("wq_t", [DIM, QDIM], F32, kind="ExternalInput").ap()
    wkv_t = nc.dram_tensor("wkv_t", [DIM, KVDIM], F32, kind="ExternalInput").ap()
    wo_t = nc.dram_tensor("wo_t", [QDIM, DIM], F32, kind="ExternalInput").ap()
    cc_d = nc.dram_tensor("cc", [S, HD], F32, kind="ExternalInput").ap()
    ss_d = nc.dram_tensor("ss", [S, HD], F32, kind="ExternalInput").ap()
    ident_d = nc.dram_tensor("ident", [128, 128], F32, kind="ExternalInput").ap()
    masks_d = nc.dram_tensor("masks", [4, 128, 512], F32, kind="ExternalInput").ap()
    out_d = nc.dram_tensor("out", [B * S, DIM], F32, kind="ExternalOutput").ap()

    with tile.TileContext(nc) as tc:
        with tc.tile_pool(name="singles", bufs=1) as singles, \
             tc.tile_pool(name="dram", bufs=1, space="DRAM") as dpool:
            ident = singles.tile([128, 128], F32R)
            nc.sync.dma_start(out=ident, in_=ident_d.bitcast(F32R))
            kt_sb = singles.tile([128, NT, 128], F32R)   # K^T: [hd, tile, tok]
            v_sb = singles.tile([128, NT, 128], F32R)    # V: [tok, tile, hd]
            qt_dram = dpool.tile([NT, NH_LOC, 128, 128], F32)

            # ---------------- phase 1: projections + rope ----------------
            with tc.tile_pool(name="p1w", bufs=1) as p1w, \
                 tc.tile_pool(name="p1", bufs=2) as p1, \
                 tc.tile_pool(name="p1r", bufs=3) as p1r, \
                 tc.tile_pool(name="ps_q", bufs=2, space="PSUM") as ps_qp, \
                 tc.tile_pool(name="ps_kv", bufs=2, space="PSUM") as ps_kvp, \
                 tc.tile_pool(name="ps_qt", bufs=2, space="PSUM") as ps_qtp, \
                 tc.tile_pool(name="ps_kt", bufs=2, space="PSUM") as ps_ktp:
                wq_sb = p1w.tile([128, NK, QDIM], F32R)
                nc.sync.dma_start(out=wq_sb,
                                  in_=wq_t.rearrange("(k p) n -> p k n", p=128).bitcast(F32R))
                wkv_sb = p1w.tile([128, NK, KVDIM], F32R)
                nc.sync.dma_start(out=wkv_sb,
                                  in_=wkv_t.rearrange("(k p) n -> p k n", p=128).bitcast(F32R))
                cc_sb = p1w.tile([128, TPB, HD], F32)
                nc.sync.dma_start(out=cc_sb, in_=cc_d.rearrange("(w p) d -> p w d", p=128))
                ss_sb = p1w.tile([128, TPB, HD], F32)
                nc.sync.dma_start(out=ss_sb, in_=ss_d.rearrange("(w p) d -> p w d", p=128))

                for tt in range(NT):
                    w = tt % TPB
                    xs = p1.tile([128, NK, 128], F32R, tag="xs")
                    nc.sync.dma_start(out=xs, in_=x_t[tt].rearrange("k p t -> p k t").bitcast(F32R))

                    ps_q = ps_qp.tile([128, QDIM], F32)
                    for k in range(NK):
                        nc.tensor.matmul(ps_q, xs[:, k, :], wq_sb[:, k, :],
                                         start=(k == 0), stop=(k == NK - 1))
                    ps_kv = ps_kvp.tile([128, KVDIM], F32)
                    for k in range(NK):
                        nc.tensor.matmul(ps_kv, xs[:, k, :], wkv_sb[:, k, :],
                                         start=(k == 0), stop=(k == NK - 1))

                    # rope Q (deinterleaved even|odd halves) + transpose per head
                    q_rope = p1r.tile([128, QDIM], F32R, tag="q_rope")
                    for h in range(NH_LOC):
                        hb = h * HD
                        sw = p1r.tile([128, HD], F32, tag="sw")
                        nc.scalar.copy(sw[:, 0:64], ps_q[:, hb + 64:hb + 128])
                        nc.scalar.copy(sw[:, 64:128], ps_q[:, hb:hb + 64])
                        t1 = p1r.tile([128, HD], F32, tag="t1")
                        nc.vector.tensor_mul(t1, ps_q[:, hb:hb + HD], cc_sb[:, w, :])
                        nc.vector.tensor_mul(sw, sw, ss_sb[:, w, :])
                        nc.vector.tensor_add(q_rope[:, hb:hb + HD], t1, sw)
                    ps_qt = ps_qtp.tile([128, QDIM], F32R)
                    for h in range(NH_LOC):
                        hb = h * HD
                        nc.tensor.transpose(ps_qt[:, hb:hb + HD], q_rope[:, hb:hb + HD], ident)
                    qt_st = p1r.tile([128, QDIM], F32R, tag="qt_st")
                    nc.scalar.copy(qt_st, ps_qt)
                    nc.sync.dma_start(
                        out=qt_dram[tt].rearrange("h p t -> p h t").bitcast(F32R),
                        in_=qt_st.rearrange("p (h t) -> p h t", h=NH_LOC))

                    # rope K + transpose; V straight copy
                    k_rope = p1r.tile([128, HD], F32R, tag="k_rope")
                    sw = p1r.tile([128, HD], F32, tag="sw")
                    nc.scalar.copy(sw[:, 0:64], ps_kv[:, 64:128])
                    nc.scalar.copy(sw[:, 64:128], ps_kv[:, 0:64])
                    t1 = p1r.tile([128, HD], F32, tag="t1")
                    nc.vector.tensor_mul(t1, ps_kv[:, 0:HD], cc_sb[:, w, :])
                    nc.vector.tensor_mul(sw, sw, ss_sb[:, w, :])
                    nc.vector.tensor_add(k_rope, t1, sw)
                    ps_kt = ps_ktp.tile([128, HD], F32R)
                    nc.tensor.transpose(ps_kt, k_rope, ident)
                    nc.scalar.copy(kt_sb[:, tt, :], ps_kt)
                    nc.scalar.copy(v_sb[:, tt, :], ps_kv[:, HD:KVDIM])

            # ---------------- phase 2/3: attention + wo ----------------
            with tc.tile_pool(name="p2w", bufs=1) as p2w, \
                 tc.tile_pool(name="p2", bufs=2) as p2, \
                 tc.tile_pool(name="p2pt", bufs=4) as p2pt, \
                 tc.tile_pool(name="p2l", bufs=4) as p2l, \
                 tc.tile_pool(name="arena_p", bufs=1) as arena_p, \
                 tc.tile_pool(name="ps_s", bufs=1, space="PSUM") as ps_sp, \
                 tc.tile_pool(name="ps_pt", bufs=2, space="PSUM") as ps_ptp, \
                 tc.tile_pool(name="ps_o", bufs=1, space="PSUM") as ps_op, \
                 tc.tile_pool(name="ps_w", bufs=1, space="PSUM") as ps_wp:
                wo_sb = p2w.tile([128, NH_LOC, DIM], F32R)
                nc.sync.dma_start(out=wo_sb,
                                  in_=wo_t.rearrange("(k p) n -> p k n", p=128).bitcast(F32R))
                masks_sb = p2w.tile([128, 4, 512], F32)
                nc.sync.dma_start(out=masks_sb, in_=masks_d.rearrange("v p n -> p v n"))

                for b in range(B):
                    for g in range(NG):
                        i0 = 2 * g
                        c = (i0 + 2 + 3) // 4        # chunks (same for both rows)
                        nt = 4 * c                   # sk tiles
                        qt_tile = p2.tile([128, 2, NH_LOC, 128], F32R, tag="qt_tile")
                        for r in range(2):
                            nc.sync.dma_start(
                                out=qt_tile[:, r],
                                in_=qt_dram[b * TPB + i0 + r].rearrange("h p t -> p h t").bitcast(F32R))
                        attn_t = p2.tile([128, NH_LOC, 2, 128], F32R, tag="attn_t")
                        for h in range(NH_LOC):
                            arena = arena_p.tile([128, 2, 4 * 512], F32R, tag="arena")
                            for r in range(2):
                                i = i0 + r
                                ps_s = ps_sp.tile([128, 4, 512], F32, tag="ps_s")
                                for ch in range(c):
                                    nc.tensor.matmul(
                                        ps_s[:, ch, :], qt_tile[:, r, h, :],
                                        kt_sb[:, b * TPB + ch * 4:b * TPB + ch * 4 + 4, :],
                                        start=True, stop=True)
                                nc.vector.tensor_add(ps_s[:, c - 1, :], ps_s[:, c - 1, :],
                                                     masks_sb[:, i % 4, :])
                                l_i = p2l.tile([128, 1], F32, tag="l_i")
                                nc.scalar.activation(
                                    arena[:, r, :c * 512], ps_s[:, :c, :],
                                    AF.Exp, scale=SOFTMAX_SCALE, accum_out=l_i)
                                rl_i = p2l.tile([128, 1], F32, tag="rl_i")
                                nc.vector.reciprocal(rl_i, l_i)
                                nc.vector.tensor_scalar_mul(
                                    arena[:, r, :c * 512], arena[:, r, :c * 512].bitcast(F32),
                                    rl_i)
                            ps_o = ps_op.tile([128, 256], F32)
                            for t in range(nt):
                                ps_pt = ps_ptp.tile([128, 256], F32R, tag="ps_pt")
                                for r in range(2):
                                    nc.tensor.transpose(
                                        ps_pt[:, r * 128:(r + 1) * 128],
                                        arena[:, r, t * 128:(t + 1) * 128], ident)
                                pt_sb = p2pt.tile([128, 256], F32R, tag="pt_sb")
                                nc.scalar.copy(pt_sb, ps_pt)
                                nc.tensor.matmul(ps_o, v_sb[:, b * TPB + t, :], pt_sb,
                                                 start=(t == 0), stop=(t == nt - 1))
                            for r in range(2):
                                nc.vector.tensor_copy(attn_t[:, h, r, :],
                                                      ps_o[:, r * 128:(r + 1) * 128])
                        # wo projection for the two token tiles
                        for r in range(2):
                            tt = b * TPB + i0 + r
                            o_sb = p2.tile([128, DIM], F32, tag="o_sb")
                            for n in range(DIM // 512):
                                ps_w = ps_wp.tile([128, 512], F32, tag="ps_w")
                                for kk in range(NH_LOC):
                                    nc.tensor.matmul(ps_w, attn_t[:, kk, r, :],
                                                     wo_sb[:, kk, n * 512:(n + 1) * 512],
                                                     start=(kk == 0), stop=(kk == NH_LOC - 1))
                                nc.scalar.copy(o_sb[:, n * 512:(n + 1) * 512], ps_w)
                            nc.sync.dma_start(out=out_d[tt * 128:(tt + 1) * 128, :], in_=o_sb)

    nc.compile()
    return nc


def host_prepare(x, wq, wk, wv, wo, freqs_cos, freqs_sin, B, S):
    """Build per-core in_maps. Weights nn.Linear-style [out, in]."""
    NT = B * S // 128
    n_heads = wq.shape[0] // HD
    n_kv = wk.shape[0] // HD
    hpc = n_heads // N_CORES       # q heads per core (4)
    kpc = n_kv // N_CORES          # kv heads per core (1)

    # deinterleave rope pairs: feature order (2i) first then (2i+1), per head
    de = np.concatenate([np.arange(0, HD, 2), np.arange(1, HD, 2)])

    xf = np.ascontiguousarray(x.reshape(B * S, DIM))
    # x^T tiled: [tt, k, p, t]
    x_t = np.ascontiguousarray(
        xf.T.reshape(NK, 128, NT, 128).transpose(2, 0, 1, 3))

    cos = np.repeat(freqs_cos, 2, axis=1)   # [S, 128] interleaved dup
    sin = np.repeat(freqs_sin, 2, axis=1)
    cc = np.ascontiguousarray(cos[:, de])                       # deinterleaved
    ss = sin.copy()
    ss[:, 0::2] *= -1.0                                         # [-sin, +sin]
    ss = np.ascontiguousarray(ss[:, de])

    ident = np.eye(128, dtype=np.float32)
    r_idx = np.arange(128)[:, None]
    j_idx = np.arange(512)[None, :]
    masks = np.stack([
        np.where(j_idx <= v * 128 + r_idx, 0.0, -1e30).astype(np.float32)
        for v in range(4)])

    in_maps = []
    for cidx in range(N_CORES):
        qs = slice(cidx * hpc * HD, (cidx + 1) * hpc * HD)
        ks = slice(cidx * kpc * HD, (cidx + 1) * kpc * HD)
        wq_c = wq[qs].reshape(hpc, HD, DIM)[:, de, :].reshape(hpc * HD, DIM)
        wk_c = wk[ks].reshape(kpc, HD, DIM)[:, de, :].reshape(kpc * HD, DIM)
        wv_c = wv[ks]
        wkv_c = np.concatenate([wk_c, wv_c], axis=0)
        wo_c = wo[:, qs]
        in_maps.append({
            "x_t": x_t,
            "wq_t": np.ascontiguousarray(wq_c.T),
            "wkv_t": np.ascontiguousarray(wkv_c.T),
            "wo_t": np.ascontiguousarray(wo_c.T),
            "cc": cc.astype(np.float32),
            "ss": ss.astype(np.float32),
            "ident": ident,
            "masks": masks,
        })
    return in_maps


_CACHE = {}


def run(inputs, trace=False, trace_cores=None):
    x = np.asarray(inputs["x"], dtype=np.float32)
    B, S, _ = x.shape
    key = (B, S)
    if key not in _CACHE:
        _CACHE[key] = build_nc(B, S)
    nc = _CACHE[key]
    in_maps = host_prepare(
        x, np.asarray(inputs["wq"], np.float32), np.asarray(inputs["wk"], np.float32),
        np.asarray(inputs["wv"], np.float32), np.asarray(inputs["wo"], np.float32),
        np.asarray(inputs["freqs_cos"], np.float32),
        np.asarray(inputs["freqs_sin"], np.float32), B, S)
    res = bass_utils.run_bass_kernel_spmd(
        nc, in_maps, core_ids=list(range(N_CORES)), trace=trace,
        trace_cores=trace_cores)
    acc = np.zeros((B * S, DIM), dtype=np.float64)
    for r in res.results:
        acc += r["out"].astype(np.float64)
    out = acc.astype(np.float32).reshape(B, S, DIM)
    return out, res


def kernel(**inputs) -> np.ndarray:
    assert int(inputs.get("start_pos", 0)) == 0
    out, _ = run(inputs, trace=False)
    return out


# revision 3
# speedup vs baseline: 1.2538x; 1.2538x over previous
"""Trainium2 Bass kernel for nn_Attention_38276748542551.

Llama-style GQA attention block (DIM=4096, 32 q-heads, 8 kv-heads, hd=128,
b=2, s=2048, start_pos=0), tensor-parallel over heads across 8 NeuronCores:
each core owns 4 q-heads / 1 kv-head (wq/wk/wv output-dim shard, wo
input-dim shard) and computes a full [b*s, 4096] partial of the wo output;
the all-reduce is done on the host after gathering the 8 partials.

All matmuls run in float32r (fp32 with 11-bit mantissa; measured 1.3e-4
rel-err over K=4096 — identical accuracy to the PE's plain-fp32 mode, 4x
faster; 232 ns/matmul at N=512 warm).

Device dataflow per core:
  phase 1 (per 128-token tile): xT tile (host pre-transposed) -> Q and K|V
  projections (contraction over DIM on partitions) -> rope (deinterleaved
  even/odd feature layout baked into the weight sharding) -> PE-transpose
  Q,K to feature-major [hd, tok]; Q spilled to a DRAM scratch, K^T and V
  stay resident in SBUF.
  phase 2 (per batch, per 512-token query block, per head): scores computed
  TRANSPOSED [sk, sq] per 128-sk-tile (lhsT = K^T tile, rhs = Q^T block,
  N=512) -> causal mask add on diagonal-band tiles -> exp (ACT) -> PV
  accumulation (lhsT = V tile, N=512) and row-sum accumulation (lhsT =
  ones column) on the PE -> softmax denominator broadcast via a K=1
  ones-matmul -> reciprocal + multiply -> attn^T, feature-major.
  phase 3 (per 128-token tile): wo projection (lhsT = attn^T, rhs = wo^T,
  8x N=512 chunks x 4 k-tiles) -> DMA partial out.
"""
import sys
import numpy as np

sys.path.insert(0, "/opt/trn_rl_repo")

import concourse.bass as bass  # noqa: E402
import concourse.tile as tile  # noqa: E402
from concourse import bacc, mybir  # noqa: E402
from concourse import bass_utils  # noqa: E402

F32 = mybir.dt.float32
F32R = mybir.dt.float32r
AF = mybir.ActivationFunctionType

DIM = 4096
NK = DIM // 128          # contraction k-tiles
HD = 128                 # head dim
NH_LOC = 4               # q heads per core
QDIM = NH_LOC * HD       # 512
KVDIM = 2 * HD           # K and V projected together, 256
N_CORES = 8
SOFTMAX_SCALE = 1.0 / np.sqrt(HD)


def build_nc(B=2, S=2048):
    """Build the per-core Bass program (identical across cores; data differs)."""
    NT = B * S // 128            # token tiles total
    TPB = S // 128               # token tiles per batch
    NQB = S // 512               # 512-wide query blocks per batch

    nc = bacc.Bacc("TRN2", target_bir_lowering=False, debug=False,
                   enable_asserts=False, num_devices=N_CORES)

    x_t = nc.dram_tensor("x_t", [NT, NK, 128, 128], F32, kind="ExternalInput").ap()
    wq_t = nc.dram_tensor("wq_t", [DIM, QDIM], F32, kind="ExternalInput").ap()
    wkv_t = nc.dram_tensor("wkv_t", [DIM, KVDIM], F32, kind="ExternalInput").ap()
    wo_t = nc.dram_tensor("wo_t", [QDIM, DIM], F32, kind="ExternalInput").ap()
    cc_d = nc.dram_tensor("cc", [S, HD], F32, kind="ExternalInput").ap()
    ss_d = nc.dram_tensor("ss", [S, HD], F32, kind="ExternalInput").ap()
    ident_d = nc.dram_tensor("ident", [128, 128], F32, kind="ExternalInput").ap()
    ones_d = nc.dram_tensor("ones", [128, 128], F32, kind="ExternalInput").ap()
    masks_d = nc.dram_tensor("masks", [4, 128, 512], F32, kind="ExternalInput").ap()
    out_d = nc.dram_tensor("out", [B * S, DIM], F32, kind="ExternalOutput").ap()

    with tile.TileContext(nc) as tc:
        with tc.tile_pool(name="singles", bufs=1) as singles, \
             tc.tile_pool(name="dram", bufs=1, space="DRAM") as dpool:
            ident = singles.tile([128, 128], F32R)
            nc.sync.dma_start(out=ident, in_=ident_d.bitcast(F32R))
            ones_sb = singles.tile([128, 128], F32R)
            nc.sync.dma_start(out=ones_sb, in_=ones_d.bitcast(F32R))
            kt_sb = singles.tile([128, NT, 128], F32R)   # K^T: [hd, tile, tok]
            v_sb = singles.tile([128, NT, 128], F32R)    # V: [tok, tile, hd]
            qt_dram = dpool.tile([NT, NH_LOC, 128, 128], F32)

            # ---------------- phase 1: projections + rope ----------------
            with tc.tile_pool(name="p1w", bufs=1) as p1w, \
                 tc.tile_pool(name="p1", bufs=2) as p1, \
                 tc.tile_pool(name="p1r", bufs=3) as p1r, \
                 tc.tile_pool(name="ps_q", bufs=2, space="PSUM") as ps_qp, \
                 tc.tile_pool(name="ps_kv", bufs=2, space="PSUM") as ps_kvp, \
                 tc.tile_pool(name="ps_qt", bufs=2, space="PSUM") as ps_qtp, \
                 tc.tile_pool(name="ps_kt", bufs=2, space="PSUM") as ps_ktp:
                wq_sb = p1w.tile([128, NK, QDIM], F32R)
                for qc in range(4):   # split load so first matmuls start early
                    ksl = slice(qc * (NK // 4) * 128, (qc + 1) * (NK // 4) * 128)
                    nc.sync.dma_start(
                        out=wq_sb[:, qc * (NK // 4):(qc + 1) * (NK // 4), :],
                        in_=wq_t[ksl, :].rearrange("(k p) n -> p k n", p=128).bitcast(F32R))
                wkv_sb = p1w.tile([128, NK, KVDIM], F32R)
                nc.sync.dma_start(out=wkv_sb,
                                  in_=wkv_t.rearrange("(k p) n -> p k n", p=128).bitcast(F32R))
                cc_sb = p1w.tile([128, TPB, HD], F32)
                nc.sync.dma_start(out=cc_sb, in_=cc_d.rearrange("(w p) d -> p w d", p=128))
                ss_sb = p1w.tile([128, TPB, HD], F32)
                nc.sync.dma_start(out=ss_sb, in_=ss_d.rearrange("(w p) d -> p w d", p=128))

                for tt in range(NT):
                    w = tt % TPB
                    xs = p1.tile([128, NK, 128], F32R, tag="xs")
                    nc.sync.dma_start(out=xs, in_=x_t[tt].rearrange("k p t -> p k t").bitcast(F32R))

                    ps_q = ps_qp.tile([128, QDIM], F32)
                    for k in range(NK):
                        nc.tensor.matmul(ps_q, xs[:, k, :], wq_sb[:, k, :],
                                         start=(k == 0), stop=(k == NK - 1))
                    ps_kv = ps_kvp.tile([128, KVDIM], F32)
                    for k in range(NK):
                        nc.tensor.matmul(ps_kv, xs[:, k, :], wkv_sb[:, k, :],
                                         start=(k == 0), stop=(k == NK - 1))

                    # rope Q (deinterleaved even|odd halves) + transpose per head
                    q_rope = p1r.tile([128, QDIM], F32R, tag="q_rope")
                    for h in range(NH_LOC):
                        hb = h * HD
                        sw = p1r.tile([128, HD], F32, tag="sw")
                        nc.scalar.copy(sw[:, 0:64], ps_q[:, hb + 64:hb + 128])
                        nc.scalar.copy(sw[:, 64:128], ps_q[:, hb:hb + 64])
                        t1 = p1r.tile([128, HD], F32, tag="t1")
                        nc.vector.tensor_mul(t1, ps_q[:, hb:hb + HD], cc_sb[:, w, :])
                        nc.vector.tensor_mul(sw, sw, ss_sb[:, w, :])
                        nc.vector.tensor_add(q_rope[:, hb:hb + HD], t1, sw)
                    ps_qt = ps_qtp.tile([128, QDIM], F32R)
                    for h in range(NH_LOC):
                        hb = h * HD
                        nc.tensor.transpose(ps_qt[:, hb:hb + HD], q_rope[:, hb:hb + HD], ident)
                    qt_st = p1r.tile([128, QDIM], F32R, tag="qt_st")
                    nc.scalar.copy(qt_st, ps_qt)
                    nc.sync.dma_start(
                        out=qt_dram[tt].rearrange("h p t -> p h t").bitcast(F32R),
                        in_=qt_st.rearrange("p (h t) -> p h t", h=NH_LOC))

                    # rope K + transpose; V straight copy
                    k_rope = p1r.tile([128, HD], F32R, tag="k_rope")
                    sw = p1r.tile([128, HD], F32, tag="sw")
                    nc.scalar.copy(sw[:, 0:64], ps_kv[:, 64:128])
                    nc.scalar.copy(sw[:, 64:128], ps_kv[:, 0:64])
                    t1 = p1r.tile([128, HD], F32, tag="t1")
                    nc.vector.tensor_mul(t1, ps_kv[:, 0:HD], cc_sb[:, w, :])
                    nc.vector.tensor_mul(sw, sw, ss_sb[:, w, :])
                    nc.vector.tensor_add(k_rope, t1, sw)
                    ps_kt = ps_ktp.tile([128, HD], F32R)
                    nc.tensor.transpose(ps_kt, k_rope, ident)
                    nc.scalar.copy(kt_sb[:, tt, :], ps_kt)
                    nc.scalar.copy(v_sb[:, tt, :], ps_kv[:, HD:KVDIM])

            # ------------- phase 2/3: attention (transposed scores) + wo -------------
            with tc.tile_pool(name="p2w", bufs=1) as p2w, \
                 tc.tile_pool(name="p2", bufs=2) as p2, \
                 tc.tile_pool(name="p2e", bufs=3) as p2e, \
                 tc.tile_pool(name="p2l", bufs=3) as p2l, \
                 tc.tile_pool(name="ps_s", bufs=2, space="PSUM") as ps_sp, \
                 tc.tile_pool(name="ps_o", bufs=2, space="PSUM") as ps_op, \
                 tc.tile_pool(name="ps_l", bufs=1, space="PSUM") as ps_lp, \
                 tc.tile_pool(name="ps_b", bufs=1, space="PSUM") as ps_bp, \
                 tc.tile_pool(name="ps_w", bufs=2, space="PSUM") as ps_wp:
                wo_sb = p2w.tile([128, NH_LOC, DIM], F32R)
                nc.sync.dma_start(out=wo_sb,
                                  in_=wo_t.rearrange("(k p) n -> p k n", p=128).bitcast(F32R))
                masks_sb = p2w.tile([128, 4, 512], F32)
                nc.sync.dma_start(out=masks_sb, in_=masks_d.rearrange("v p n -> p v n"))

                for b in range(B):
                    for qb in range(NQB):
                        nt = 4 * (qb + 1)            # sk tiles for this block
                        qt_tile = p2.tile([128, NH_LOC, 4, 128], F32R, tag="qt_tile")
                        for r in range(4):
                            nc.sync.dma_start(
                                out=qt_tile[:, :, r, :],
                                in_=qt_dram[b * TPB + qb * 4 + r].rearrange("h p t -> p h t").bitcast(F32R))
                        attn_t = p2.tile([128, NH_LOC, 4, 128], F32R, tag="attn_t")
                        for h in range(NH_LOC):
                            ps_o = ps_op.tile([128, 512], F32, tag="ps_o")
                            ps_l = ps_lp.tile([1, 512], F32, tag="ps_l")
                            for t in range(nt):
                                ps_s = ps_sp.tile([128, 512], F32, tag="ps_s")
                                nc.tensor.matmul(ps_s, kt_sb[:, b * TPB + t, :],
                                                 qt_tile[:, h].rearrange("p r t -> p (r t)"),
                                                 start=True, stop=True)
                                v = t - 4 * qb
                                if v >= 0:   # diagonal band: causal mask
                                    nc.vector.tensor_add(ps_s, ps_s, masks_sb[:, v, :])
                                et = p2e.tile([128, 512], F32R, tag="et")
                                nc.scalar.activation(et, ps_s, AF.Exp, scale=SOFTMAX_SCALE)
                                nc.tensor.matmul(ps_o, v_sb[:, b * TPB + t, :], et,
                                                 start=(t == 0), stop=(t == nt - 1))
                                nc.tensor.matmul(ps_l, ones_sb[:, 0:1], et,
                                                 start=(t == 0), stop=(t == nt - 1))
                            lr = p2l.tile([1, 512], F32R, tag="lr")
                            nc.scalar.copy(lr, ps_l)
                            ps_b = ps_bp.tile([128, 512], F32, tag="ps_b")
                            nc.tensor.matmul(ps_b, ones_sb[0:1, :], lr, start=True, stop=True)
                            rb = p2l.tile([128, 512], F32, tag="rb")
                            nc.vector.reciprocal(rb, ps_b)
                            nc.vector.tensor_mul(
                                attn_t[:, h].rearrange("p r t -> p (r t)"), ps_o, rb)
                        # wo projection for the four token tiles
                        for r in range(4):
                            tt = b * TPB + qb * 4 + r
                            o_sb = p2.tile([128, DIM], F32, tag="o_sb")
                            for n in range(DIM // 512):
                                ps_w = ps_wp.tile([128, 512], F32, tag="ps_w")
                                for kk in range(NH_LOC):
                                    nc.tensor.matmul(ps_w, attn_t[:, kk, r, :],
                                                     wo_sb[:, kk, n * 512:(n + 1) * 512],
                                                     start=(kk == 0), stop=(kk == NH_LOC - 1))
                                nc.scalar.copy(o_sb[:, n * 512:(n + 1) * 512], ps_w)
                            nc.sync.dma_start(out=out_d[tt * 128:(tt + 1) * 128, :], in_=o_sb)

    nc.compile()
    return nc


def host_prepare(x, wq, wk, wv, wo, freqs_cos, freqs_sin, B, S):
    """Build per-core in_maps. Weights nn.Linear-style [out, in]."""
    NT = B * S // 128
    n_heads = wq.shape[0] // HD
    n_kv = wk.shape[0] // HD
    hpc = n_heads // N_CORES       # q heads per core (4)
    kpc = n_kv // N_CORES          # kv heads per core (1)

    # deinterleave rope pairs: feature order (2i) first then (2i+1), per head
    de = np.concatenate([np.arange(0, HD, 2), np.arange(1, HD, 2)])

    xf = np.ascontiguousarray(x.reshape(B * S, DIM))
    # x^T tiled: [tt, k, p, t]
    x_t = np.ascontiguousarray(
        xf.T.reshape(NK, 128, NT, 128).transpose(2, 0, 1, 3))

    cos = np.repeat(freqs_cos, 2, axis=1)   # [S, 128] interleaved dup
    sin = np.repeat(freqs_sin, 2, axis=1)
    cc = np.ascontiguousarray(cos[:, de])                       # deinterleaved
    ss = sin.copy()
    ss[:, 0::2] *= -1.0                                         # [-sin, +sin]
    ss = np.ascontiguousarray(ss[:, de])

    ident = np.eye(128, dtype=np.float32)
    ones = np.ones((128, 128), dtype=np.float32)
    # transposed-orientation causal masks: scores^T [sk within tile, sq in 512]
    # variant v = t - 4*qb: allowed when sk_global <= sq_global:
    # (v*128 + r) <= j  for row r (sk), col j (sq)
    r_idx = np.arange(128)[:, None]
    j_idx = np.arange(512)[None, :]
    masks = np.stack([
        np.where(v * 128 + r_idx <= j_idx, 0.0, -1e30).astype(np.float32)
        for v in range(4)])

    in_maps = []
    for cidx in range(N_CORES):
        qs = slice(cidx * hpc * HD, (cidx + 1) * hpc * HD)
        ks = slice(cidx * kpc * HD, (cidx + 1) * kpc * HD)
        wq_c = wq[qs].reshape(hpc, HD, DIM)[:, de, :].reshape(hpc * HD, DIM)
        wk_c = wk[ks].reshape(kpc, HD, DIM)[:, de, :].reshape(kpc * HD, DIM)
        wv_c = wv[ks]
        wkv_c = np.concatenate([wk_c, wv_c], axis=0)
        wo_c = wo[:, qs]
        in_maps.append({
            "x_t": x_t,
            "wq_t": np.ascontiguousarray(wq_c.T),
            "wkv_t": np.ascontiguousarray(wkv_c.T),
            "wo_t": np.ascontiguousarray(wo_c.T),
            "cc": cc.astype(np.float32),
            "ss": ss.astype(np.float32),
            "ident": ident,
            "ones": ones,
            "masks": masks,
        })
    return in_maps


_CACHE = {}


def run(inputs, trace=False, trace_cores=None):
    x = np.asarray(inputs["x"], dtype=np.float32)
    B, S, _ = x.shape
    key = (B, S)
    if key not in _CACHE:
        _CACHE[key] = build_nc(B, S)
    nc = _CACHE[key]
    in_maps = host_prepare(
        x, np.asarray(inputs["wq"], np.float32), np.asarray(inputs["wk"], np.float32),
        np.asarray(inputs["wv"], np.float32), np.asarray(inputs["wo"], np.float32),
        np.asarray(inputs["freqs_cos"], np.float32),
        np.asarray(inputs["freqs_sin"], np.float32), B, S)
    res = bass_utils.run_bass_kernel_spmd(
        nc, in_maps, core_ids=list(range(N_CORES)), trace=trace,
        trace_cores=trace_cores)
    acc = np.zeros((B * S, DIM), dtype=np.float64)
    for r in res.results:
        acc += r["out"].astype(np.float64)
    out = acc.astype(np.float32).reshape(B, S, DIM)
    return out, res


def kernel(**inputs) -> np.ndarray:
    assert int(inputs.get("start_pos", 0)) == 0
    out, _ = run(inputs, trace=False)
    return out


# revision 6
# speedup vs baseline: 1.3756x; 1.0971x over previous
"""Trainium2 Bass kernel for nn_Attention_38276748542551.

Llama-style GQA attention block (DIM=4096, 32 q-heads, 8 kv-heads, hd=128,
b=2, s=2048, start_pos=0), tensor-parallel over heads across 8 NeuronCores:
each core owns 4 q-heads / 1 kv-head (wq/wk/wv output-dim shard, wo
input-dim shard) and computes a full [b*s, 4096] partial of the wo output;
the all-reduce is done on the host after gathering the 8 partials.

All matmuls run in float32r (fp32 with 11-bit mantissa; measured 1.3e-4
rel-err over K=4096 — identical accuracy to the PE's plain-fp32 mode, 4x
faster; 232 ns/matmul at N=512 warm).

Device dataflow per core:
  phase 1 (per 128-token tile): xT tile (host pre-transposed) -> Q and K|V
  projections (contraction over DIM on partitions) -> rope (deinterleaved
  even/odd feature layout baked into the weight sharding) -> PE-transpose
  Q,K to feature-major [hd, tok]; Q spilled to a DRAM scratch, K^T and V
  stay resident in SBUF.
  phase 2 (per batch, per 512-token query block, per head): scores computed
  TRANSPOSED [sk, sq] per 128-sk-tile (lhsT = K^T tile, rhs = Q^T block,
  N=512) -> causal mask add on diagonal-band tiles -> exp (ACT) -> PV
  accumulation (lhsT = V tile, N=512) and row-sum accumulation (lhsT =
  ones column) on the PE -> softmax denominator broadcast via a K=1
  ones-matmul -> reciprocal + multiply -> attn^T, feature-major.
  phase 3 (per 128-token tile): wo projection (lhsT = attn^T, rhs = wo^T,
  8x N=512 chunks x 4 k-tiles) -> DMA partial out.
"""
import sys
import numpy as np

sys.path.insert(0, "/opt/trn_rl_repo")

import concourse.bass as bass  # noqa: E402
import concourse.tile as tile  # noqa: E402
from concourse import bacc, mybir  # noqa: E402
from concourse import bass_utils  # noqa: E402

F32 = mybir.dt.float32
F32R = mybir.dt.float32r
AF = mybir.ActivationFunctionType

DIM = 4096
NK = DIM // 128          # contraction k-tiles
HD = 128                 # head dim
NH_LOC = 4               # q heads per core
QDIM = NH_LOC * HD       # 512
KVDIM = 2 * HD           # K and V projected together, 256
N_CORES = 8
SOFTMAX_SCALE = 1.0 / np.sqrt(HD)


def build_nc(B=2, S=2048):
    """Build the per-core Bass program (identical across cores; data differs)."""
    NT = B * S // 128            # token tiles total
    TPB = S // 128               # token tiles per batch
    NQB = S // 512               # 512-wide query blocks per batch

    nc = bacc.Bacc("TRN2", target_bir_lowering=False, debug=False,
                   enable_asserts=False, num_devices=N_CORES)

    x_t = nc.dram_tensor("x_t", [NT, NK, 128, 128], F32, kind="ExternalInput").ap()
    wq_t = nc.dram_tensor("wq_t", [DIM, QDIM], F32, kind="ExternalInput").ap()
    wkv_t = nc.dram_tensor("wkv_t", [DIM, KVDIM], F32, kind="ExternalInput").ap()
    wo_t = nc.dram_tensor("wo_t", [QDIM, DIM], F32, kind="ExternalInput").ap()
    cc_d = nc.dram_tensor("cc", [S, HD], F32, kind="ExternalInput").ap()
    ss_d = nc.dram_tensor("ss", [S, HD], F32, kind="ExternalInput").ap()
    ident_d = nc.dram_tensor("ident", [128, 128], F32, kind="ExternalInput").ap()
    ones_d = nc.dram_tensor("ones", [128, 128], F32, kind="ExternalInput").ap()
    masks_d = nc.dram_tensor("masks", [4, 128, 512], F32, kind="ExternalInput").ap()
    out_d = nc.dram_tensor("out", [B * S, DIM], F32, kind="ExternalOutput").ap()

    with tile.TileContext(nc) as tc:
        with tc.tile_pool(name="singles", bufs=1) as singles, \
             tc.tile_pool(name="dram", bufs=1, space="DRAM") as dpool:
            ident = singles.tile([128, 128], F32R)
            nc.sync.dma_start(out=ident, in_=ident_d.bitcast(F32R))
            ones_sb = singles.tile([128, 128], F32R)
            nc.sync.dma_start(out=ones_sb, in_=ones_d.bitcast(F32R))
            kt_sb = singles.tile([128, NT, 128], F32R)   # K^T: [hd, tile, tok]
            v_sb = singles.tile([128, NT, 128], F32R)    # V: [tok, tile, hd]
            qt_dram = dpool.tile([NT, NH_LOC, 128, 128], F32)

            # ---------------- phase 1: projections + rope ----------------
            with tc.tile_pool(name="p1w", bufs=1) as p1w, \
                 tc.tile_pool(name="p1", bufs=2) as p1, \
                 tc.tile_pool(name="p1r", bufs=3) as p1r, \
                 tc.tile_pool(name="ps_q", bufs=2, space="PSUM") as ps_qp, \
                 tc.tile_pool(name="ps_kv", bufs=2, space="PSUM") as ps_kvp, \
                 tc.tile_pool(name="ps_qt", bufs=2, space="PSUM") as ps_qtp, \
                 tc.tile_pool(name="ps_kt", bufs=2, space="PSUM") as ps_ktp:
                def load_xs(tt):
                    t_ = p1.tile([128, NK, 128], F32R, tag="xs")
                    nc.sync.dma_start(out=t_, in_=x_t[tt].rearrange("k p t -> p k t").bitcast(F32R))
                    return t_

                xs_next = load_xs(0)   # first activation tile before the weights
                wq_sb = p1w.tile([128, NK, QDIM], F32R)
                for qc in range(4):   # split load so first matmuls start early
                    ksl = slice(qc * (NK // 4) * 128, (qc + 1) * (NK // 4) * 128)
                    nc.sync.dma_start(
                        out=wq_sb[:, qc * (NK // 4):(qc + 1) * (NK // 4), :],
                        in_=wq_t[ksl, :].rearrange("(k p) n -> p k n", p=128).bitcast(F32R))
                wkv_sb = p1w.tile([128, NK, KVDIM], F32R)
                nc.sync.dma_start(out=wkv_sb,
                                  in_=wkv_t.rearrange("(k p) n -> p k n", p=128).bitcast(F32R))
                cc_sb = p1w.tile([128, TPB, HD], F32)
                nc.sync.dma_start(out=cc_sb, in_=cc_d.rearrange("(w p) d -> p w d", p=128))
                ss_sb = p1w.tile([128, TPB, HD], F32)
                nc.sync.dma_start(out=ss_sb, in_=ss_d.rearrange("(w p) d -> p w d", p=128))

                for tt in range(NT):
                    w = tt % TPB
                    xs = xs_next
                    if tt + 1 < NT:
                        xs_next = load_xs(tt + 1)

                    ps_q = ps_qp.tile([128, QDIM], F32)
                    for k in range(NK):
                        nc.tensor.matmul(ps_q, xs[:, k, :], wq_sb[:, k, :],
                                         start=(k == 0), stop=(k == NK - 1))
                    ps_kv = ps_kvp.tile([128, KVDIM], F32)
                    for k in range(NK):
                        nc.tensor.matmul(ps_kv, xs[:, k, :], wkv_sb[:, k, :],
                                         start=(k == 0), stop=(k == NK - 1))

                    # rope Q (deinterleaved even|odd halves) + transpose per head
                    q_rope = p1r.tile([128, QDIM], F32R, tag="q_rope")
                    for h in range(NH_LOC):
                        hb = h * HD
                        sw = p1r.tile([128, HD], F32, tag="sw")
                        nc.scalar.copy(sw[:, 0:64], ps_q[:, hb + 64:hb + 128])
                        nc.scalar.copy(sw[:, 64:128], ps_q[:, hb:hb + 64])
                        t1 = p1r.tile([128, HD], F32, tag="t1")
                        nc.vector.tensor_mul(t1, ps_q[:, hb:hb + HD], cc_sb[:, w, :])
                        nc.vector.tensor_mul(sw, sw, ss_sb[:, w, :])
                        nc.vector.tensor_add(q_rope[:, hb:hb + HD], t1, sw)
                    ps_qt = ps_qtp.tile([128, QDIM], F32R)
                    for h in range(NH_LOC):
                        hb = h * HD
                        nc.tensor.transpose(ps_qt[:, hb:hb + HD], q_rope[:, hb:hb + HD], ident)
                    qt_st = p1r.tile([128, QDIM], F32R, tag="qt_st")
                    nc.scalar.copy(qt_st, ps_qt)
                    nc.sync.dma_start(
                        out=qt_dram[tt].rearrange("h p t -> p h t").bitcast(F32R),
                        in_=qt_st.rearrange("p (h t) -> p h t", h=NH_LOC))

                    # rope K + transpose; V straight copy
                    k_rope = p1r.tile([128, HD], F32R, tag="k_rope")
                    sw = p1r.tile([128, HD], F32, tag="sw")
                    nc.scalar.copy(sw[:, 0:64], ps_kv[:, 64:128])
                    nc.scalar.copy(sw[:, 64:128], ps_kv[:, 0:64])
                    t1 = p1r.tile([128, HD], F32, tag="t1")
                    nc.vector.tensor_mul(t1, ps_kv[:, 0:HD], cc_sb[:, w, :])
                    nc.vector.tensor_mul(sw, sw, ss_sb[:, w, :])
                    nc.vector.tensor_add(k_rope, t1, sw)
                    ps_kt = ps_ktp.tile([128, HD], F32R)
                    nc.tensor.transpose(ps_kt, k_rope, ident)
                    nc.scalar.copy(kt_sb[:, tt, :], ps_kt)
                    nc.scalar.copy(v_sb[:, tt, :], ps_kv[:, HD:KVDIM])

            # ------------- phase 2/3: attention (transposed scores) + wo -------------
            with tc.tile_pool(name="p2w", bufs=1) as p2w, \
                 tc.tile_pool(name="p2", bufs=2) as p2, \
                 tc.tile_pool(name="p2e", bufs=3) as p2e, \
                 tc.tile_pool(name="p2l", bufs=3) as p2l, \
                 tc.tile_pool(name="ps_s", bufs=2, space="PSUM") as ps_sp, \
                 tc.tile_pool(name="ps_o", bufs=2, space="PSUM") as ps_op, \
                 tc.tile_pool(name="ps_l", bufs=1, space="PSUM") as ps_lp, \
                 tc.tile_pool(name="ps_b", bufs=1, space="PSUM") as ps_bp, \
                 tc.tile_pool(name="ps_w", bufs=2, space="PSUM") as ps_wp:
                masks_sb = p2w.tile([128, 4, 512], F32)
                nc.sync.dma_start(out=masks_sb, in_=masks_d.rearrange("v p n -> p v n"))
                wo_sb = p2w.tile([128, NH_LOC, DIM], F32R)
                for kk in range(NH_LOC):   # chunked so the first wo can start early
                    nc.sync.dma_start(
                        out=wo_sb[:, kk, :],
                        in_=wo_t[kk * 128:(kk + 1) * 128, :].bitcast(F32R))

                for b in range(B):
                    for qb in range(NQB):
                        nt = 4 * (qb + 1)            # sk tiles for this block
                        qt_tile = p2.tile([128, NH_LOC, 4, 128], F32R, tag="qt_tile")
                        for r in range(4):
                            nc.sync.dma_start(
                                out=qt_tile[:, :, r, :],
                                in_=qt_dram[b * TPB + qb * 4 + r].rearrange("h p t -> p h t").bitcast(F32R))
                        attn_t = p2.tile([128, NH_LOC, 4, 128], F32R, tag="attn_t")
                        for h in range(NH_LOC):
                            ps_o = ps_op.tile([128, 512], F32, tag="ps_o")
                            ps_l = ps_lp.tile([1, 512], F32, tag="ps_l")
                            for t in range(nt):
                                ps_s = ps_sp.tile([128, 512], F32, tag="ps_s")
                                nc.tensor.matmul(ps_s, kt_sb[:, b * TPB + t, :],
                                                 qt_tile[:, h].rearrange("p r t -> p (r t)"),
                                                 start=True, stop=True)
                                v = t - 4 * qb
                                if v >= 0:   # diagonal band: causal mask
                                    nc.vector.tensor_add(ps_s, ps_s, masks_sb[:, v, :])
                                et = p2e.tile([128, 512], F32R, tag="et")
                                nc.scalar.activation(et, ps_s, AF.Exp, scale=SOFTMAX_SCALE)
                                nc.tensor.matmul(ps_o, v_sb[:, b * TPB + t, :], et,
                                                 start=(t == 0), stop=(t == nt - 1))
                                nc.tensor.matmul(ps_l, ones_sb[:, 0:1], et,
                                                 start=(t == 0), stop=(t == nt - 1))
                            lr = p2l.tile([1, 512], F32R, tag="lr")
                            nc.scalar.copy(lr, ps_l)
                            ps_b = ps_bp.tile([128, 512], F32, tag="ps_b")
                            nc.tensor.matmul(ps_b, ones_sb[0:1, :], lr, start=True, stop=True)
                            rb = p2l.tile([128, 512], F32, tag="rb")
                            nc.vector.reciprocal_approx_fast(out=rb, in_=ps_b)
                            nc.vector.tensor_mul(
                                attn_t[:, h].rearrange("p r t -> p (r t)"), ps_o, rb)
                        # wo projection for the four token tiles
                        for r in range(4):
                            tt = b * TPB + qb * 4 + r
                            o_sb = p2.tile([128, DIM], F32, tag="o_sb")
                            for n in range(DIM // 512):
                                ps_w = ps_wp.tile([128, 512], F32, tag="ps_w")
                                for kk in range(NH_LOC):
                                    nc.tensor.matmul(ps_w, attn_t[:, kk, r, :],
                                                     wo_sb[:, kk, n * 512:(n + 1) * 512],
                                                     start=(kk == 0), stop=(kk == NH_LOC - 1))
                                nc.scalar.copy(o_sb[:, n * 512:(n + 1) * 512], ps_w)
                            nc.sync.dma_start(out=out_d[tt * 128:(tt + 1) * 128, :], in_=o_sb)

    nc.compile()
    return nc


def host_prepare(x, wq, wk, wv, wo, freqs_cos, freqs_sin, B, S):
    """Build per-core in_maps. Weights nn.Linear-style [out, in]."""
    NT = B * S // 128
    n_heads = wq.shape[0] // HD
    n_kv = wk.shape[0] // HD
    hpc = n_heads // N_CORES       # q heads per core (4)
    kpc = n_kv // N_CORES          # kv heads per core (1)

    # deinterleave rope pairs: feature order (2i) first then (2i+1), per head
    de = np.concatenate([np.arange(0, HD, 2), np.arange(1, HD, 2)])

    xf = np.ascontiguousarray(x.reshape(B * S, DIM))
    # x^T tiled: [tt, k, p, t]
    x_t = np.ascontiguousarray(
        xf.T.reshape(NK, 128, NT, 128).transpose(2, 0, 1, 3))

    cos = np.repeat(freqs_cos, 2, axis=1)   # [S, 128] interleaved dup
    sin = np.repeat(freqs_sin, 2, axis=1)
    cc = np.ascontiguousarray(cos[:, de])                       # deinterleaved
    ss = sin.copy()
    ss[:, 0::2] *= -1.0                                         # [-sin, +sin]
    ss = np.ascontiguousarray(ss[:, de])

    ident = np.eye(128, dtype=np.float32)
    ones = np.ones((128, 128), dtype=np.float32)
    # transposed-orientation causal masks: scores^T [sk within tile, sq in 512]
    # variant v = t - 4*qb: allowed when sk_global <= sq_global:
    # (v*128 + r) <= j  for row r (sk), col j (sq)
    r_idx = np.arange(128)[:, None]
    j_idx = np.arange(512)[None, :]
    masks = np.stack([
        np.where(v * 128 + r_idx <= j_idx, 0.0, -1e30).astype(np.float32)
        for v in range(4)])

    in_maps = []
    for cidx in range(N_CORES):
        qs = slice(cidx * hpc * HD, (cidx + 1) * hpc * HD)
        ks = slice(cidx * kpc * HD, (cidx + 1) * kpc * HD)
        wq_c = wq[qs].reshape(hpc, HD, DIM)[:, de, :].reshape(hpc * HD, DIM)
        wk_c = wk[ks].reshape(kpc, HD, DIM)[:, de, :].reshape(kpc * HD, DIM)
        wv_c = wv[ks]
        wkv_c = np.concatenate([wk_c, wv_c], axis=0)
        wo_c = wo[:, qs]
        in_maps.append({
            "x_t": x_t,
            "wq_t": np.ascontiguousarray(wq_c.T),
            "wkv_t": np.ascontiguousarray(wkv_c.T),
            "wo_t": np.ascontiguousarray(wo_c.T),
            "cc": cc.astype(np.float32),
            "ss": ss.astype(np.float32),
            "ident": ident,
            "ones": ones,
            "masks": masks,
        })
    return in_maps


_CACHE = {}


def run(inputs, trace=False, trace_cores=None):
    x = np.asarray(inputs["x"], dtype=np.float32)
    B, S, _ = x.shape
    key = (B, S)
    if key not in _CACHE:
        _CACHE[key] = build_nc(B, S)
    nc = _CACHE[key]
    in_maps = host_prepare(
        x, np.asarray(inputs["wq"], np.float32), np.asarray(inputs["wk"], np.float32),
        np.asarray(inputs["wv"], np.float32), np.asarray(inputs["wo"], np.float32),
        np.asarray(inputs["freqs_cos"], np.float32),
        np.asarray(inputs["freqs_sin"], np.float32), B, S)
    res = bass_utils.run_bass_kernel_spmd(
        nc, in_maps, core_ids=list(range(N_CORES)), trace=trace,
        trace_cores=trace_cores)
    acc = np.zeros((B * S, DIM), dtype=np.float64)
    for r in res.results:
        acc += r["out"].astype(np.float64)
    out = acc.astype(np.float32).reshape(B, S, DIM)
    return out, res


def kernel(**inputs) -> np.ndarray:
    assert int(inputs.get("start_pos", 0)) == 0
    out, _ = run(inputs, trace=False)
    return out


# revision 8
# speedup vs baseline: 1.4014x; 1.0188x over previous
"""Trainium2 Bass kernel for nn_Attention_38276748542551.

Llama-style GQA attention block (DIM=4096, 32 q-heads, 8 kv-heads, hd=128,
b=2, s=2048, start_pos=0), tensor-parallel over heads across 8 NeuronCores:
each core owns 4 q-heads / 1 kv-head (wq/wk/wv output-dim shard, wo
input-dim shard) and computes a full [b*s, 4096] partial of the wo output;
the all-reduce is done on the host after gathering the 8 partials.

All matmuls run in float32r (fp32 with 11-bit mantissa; measured 1.3e-4
rel-err over K=4096 — same accuracy as the PE's plain-fp32 mode, 4x faster;
232 ns/matmul at N=512 warm).

Device dataflow per core (everything feature-major, moving dim = 512 tokens):
  phase 1 (per 512-token block): Q/K/V projections with the weight k-tile
  stationary and x^T (host pre-transposed) moving -> outputs land [feat, tok]
  -> rope applied in feature-major form: pair-swap is a 128x128 permutation
  matmul (the even/odd deinterleave is baked into the weight sharding, so the
  swap is a partition half-rotation), then two multiplies + add against
  transposed rope tables -> Q^T spilled to DRAM scratch, K^T resident in
  SBUF, V PE-transposed back to token-major and resident in SBUF.
  phase 2 (per batch, per 512-token query block, per head): scores computed
  TRANSPOSED [sk, sq] per 128-sk-tile (lhsT = K^T tile, rhs = Q^T block,
  N=512) -> causal mask add on diagonal-band tiles -> exp (ACT) -> PV
  accumulation (lhsT = V tile, N=512) and row-sum accumulation (lhsT = ones
  column) on the PE -> denominator broadcast via a K=1 ones-matmul ->
  approx-reciprocal + multiply -> attn^T, feature-major.
  phase 3 (per 128-token tile): wo projection (lhsT = attn^T, rhs = wo^T,
  8x N=512 chunks x 4 k-tiles) -> DMA partial out.
"""
import sys
import numpy as np

sys.path.insert(0, "/opt/trn_rl_repo")

import concourse.bass as bass  # noqa: E402
import concourse.tile as tile  # noqa: E402
from concourse import bacc, mybir  # noqa: E402
from concourse import bass_utils  # noqa: E402

F32 = mybir.dt.float32
F32R = mybir.dt.float32r
AF = mybir.ActivationFunctionType

DIM = 4096
NK = DIM // 128          # contraction k-tiles (32)
NKQ = 4                  # k quarters
KPQ = NK // NKQ          # k-tiles per quarter (8)
HD = 128                 # head dim
NH_LOC = 4               # q heads per core
QDIM = NH_LOC * HD       # 512
KVDIM = 2 * HD           # K and V projected together, 256
N_CORES = 8
SOFTMAX_SCALE = 1.0 / np.sqrt(HD)


def build_nc(B=2, S=2048):
    """Build the per-core Bass program (identical across cores; data differs)."""
    NT = B * S // 128            # 128-token tiles total
    TPB = S // 128               # 128-token tiles per batch
    NQB = S // 512               # 512-token blocks per batch
    NTG = B * NQB                # 512-token blocks total

    nc = bacc.Bacc("TRN2", target_bir_lowering=False, debug=False,
                   enable_asserts=False, num_devices=N_CORES)

    x_t = nc.dram_tensor("x_t", [NTG, NKQ, KPQ, 128, 512], F32, kind="ExternalInput").ap()
    wq_t = nc.dram_tensor("wq_t", [DIM, QDIM], F32, kind="ExternalInput").ap()
    wkv_t = nc.dram_tensor("wkv_t", [DIM, KVDIM], F32, kind="ExternalInput").ap()
    wo_t = nc.dram_tensor("wo_t", [QDIM, DIM], F32, kind="ExternalInput").ap()
    cct_d = nc.dram_tensor("cct", [128, S], F32, kind="ExternalInput").ap()
    sst_d = nc.dram_tensor("sst", [128, S], F32, kind="ExternalInput").ap()
    ident_d = nc.dram_tensor("ident", [128, 128], F32, kind="ExternalInput").ap()
    ones_d = nc.dram_tensor("ones", [128, 128], F32, kind="ExternalInput").ap()
    swap_d = nc.dram_tensor("swap", [128, 128], F32, kind="ExternalInput").ap()
    masks_d = nc.dram_tensor("masks", [4, 128, 512], F32, kind="ExternalInput").ap()
    out_d = nc.dram_tensor("out", [B * S, DIM], F32, kind="ExternalOutput").ap()

    with tile.TileContext(nc) as tc:
        with tc.tile_pool(name="singles", bufs=1) as singles, \
             tc.tile_pool(name="dram", bufs=1, space="DRAM") as dpool:
            ident = singles.tile([128, 128], F32R)
            nc.sync.dma_start(out=ident, in_=ident_d.bitcast(F32R))
            ones_sb = singles.tile([128, 128], F32R)
            nc.sync.dma_start(out=ones_sb, in_=ones_d.bitcast(F32R))
            swap_sb = singles.tile([128, 128], F32R)
            nc.sync.dma_start(out=swap_sb, in_=swap_d.bitcast(F32R))
            kt_sb = singles.tile([128, NT, 128], F32R)   # K^T: [hd, tile, tok]
            v_sb = singles.tile([128, NT, 128], F32R)    # V: [tok, tile, hd]
            qt_dram = dpool.tile([NTG, NH_LOC, 128, 512], F32)

            # ---------------- phase 1: projections + rope (feature-major) ----------------
            with tc.tile_pool(name="p1w", bufs=1) as p1w, \
                 tc.tile_pool(name="p1", bufs=2) as p1, \
                 tc.tile_pool(name="p1r", bufs=3) as p1r, \
                 tc.tile_pool(name="p1t", bufs=2) as p1t, \
                 tc.tile_pool(name="ps_acc", bufs=6, space="PSUM") as ps_accp, \
                 tc.tile_pool(name="ps_misc", bufs=2, space="PSUM") as ps_miscp:

                def load_xs(g, kq):
                    t_ = p1.tile([128, KPQ, 512], F32R, tag="xs")
                    nc.sync.dma_start(
                        out=t_, in_=x_t[g, kq].rearrange("k p t -> p k t").bitcast(F32R))
                    return t_

                xs_next = load_xs(0, 0)
                wq_sb = p1w.tile([128, NK, QDIM], F32R)
                for qc in range(4):   # split load so first matmuls start early
                    ksl = slice(qc * (NK // 4) * 128, (qc + 1) * (NK // 4) * 128)
                    nc.sync.dma_start(
                        out=wq_sb[:, qc * (NK // 4):(qc + 1) * (NK // 4), :],
                        in_=wq_t[ksl, :].rearrange("(k p) n -> p k n", p=128).bitcast(F32R))
                wkv_sb = p1w.tile([128, NK, KVDIM], F32R)
                nc.sync.dma_start(out=wkv_sb,
                                  in_=wkv_t.rearrange("(k p) n -> p k n", p=128).bitcast(F32R))

                for g in range(NTG):
                    # per-block rope table slices [128, 512]
                    pos = (g % NQB) * 512
                    cct = p1t.tile([128, 512], F32, tag="cct")
                    nc.sync.dma_start(out=cct, in_=cct_d[:, pos:pos + 512])
                    sst = p1t.tile([128, 512], F32, tag="sst")
                    nc.sync.dma_start(out=sst, in_=sst_d[:, pos:pos + 512])

                    acc = [ps_accp.tile([128, 512], F32, tag="acc", name=f"acc{g}_{j}") for j in range(6)]
                    for kq in range(NKQ):
                        xs = xs_next
                        if g * NKQ + kq + 1 < NTG * NKQ:
                            nxt = g * NKQ + kq + 1
                            xs_next = load_xs(nxt // NKQ, nxt % NKQ)
                        for k in range(KPQ):
                            kt = kq * KPQ + k
                            st = (kt == 0)
                            sp = (kt == NK - 1)
                            for h in range(NH_LOC):
                                nc.tensor.matmul(acc[h], wq_sb[:, kt, h * 128:(h + 1) * 128],
                                                 xs[:, k, :], start=st, stop=sp)
                            nc.tensor.matmul(acc[4], wkv_sb[:, kt, 0:128],
                                             xs[:, k, :], start=st, stop=sp)
                            nc.tensor.matmul(acc[5], wkv_sb[:, kt, 128:256],
                                             xs[:, k, :], start=st, stop=sp)

                    # rope Q (4 heads) + K in feature-major; V transpose to token-major
                    for j in range(5):   # 0..3 = q heads, 4 = K
                        f = p1r.tile([128, 512], F32R, tag="f")
                        nc.scalar.copy(f, acc[j])
                        ps_sw = ps_miscp.tile([128, 512], F32, tag="misc")
                        nc.tensor.matmul(ps_sw, swap_sb, f, start=True, stop=True)
                        t1 = p1r.tile([128, 512], F32, tag="t1")
                        nc.vector.tensor_mul(t1, f.bitcast(F32), cct)
                        t2 = p1r.tile([128, 512], F32, tag="t2")
                        nc.vector.tensor_mul(t2, ps_sw, sst)
                        if j < NH_LOC:
                            qr = p1r.tile([128, 512], F32R, tag="qr")
                            nc.vector.tensor_add(qr, t1, t2)
                            nc.sync.dma_start(out=qt_dram[g, j].bitcast(F32R), in_=qr)
                        else:
                            nc.vector.tensor_add(
                                kt_sb[:, 4 * g:4 * g + 4, :].rearrange("p a t -> p (a t)"),
                                t1, t2)
                    vf = p1r.tile([128, 512], F32R, tag="f")
                    nc.scalar.copy(vf, acc[5])
                    for r in range(4):
                        ps_vt = ps_miscp.tile([128, 512], F32R, tag="misc")
                        nc.tensor.transpose(ps_vt[:, 0:128], vf[:, r * 128:(r + 1) * 128], ident)
                        nc.scalar.copy(v_sb[:, 4 * g + r, :], ps_vt[:, 0:128])

            # ------------- phase 2/3: attention (transposed scores) + wo -------------
            with tc.tile_pool(name="p2w", bufs=1) as p2w, \
                 tc.tile_pool(name="p2", bufs=2) as p2, \
                 tc.tile_pool(name="p2e", bufs=3) as p2e, \
                 tc.tile_pool(name="p2l", bufs=3) as p2l, \
                 tc.tile_pool(name="ps_s", bufs=2, space="PSUM") as ps_sp, \
                 tc.tile_pool(name="ps_o", bufs=2, space="PSUM") as ps_op, \
                 tc.tile_pool(name="ps_l", bufs=1, space="PSUM") as ps_lp, \
                 tc.tile_pool(name="ps_b", bufs=1, space="PSUM") as ps_bp, \
                 tc.tile_pool(name="ps_w", bufs=2, space="PSUM") as ps_wp:
                masks_sb = p2w.tile([128, 4, 512], F32)
                nc.sync.dma_start(out=masks_sb, in_=masks_d.rearrange("v p n -> p v n"))
                wo_sb = p2w.tile([128, NH_LOC, DIM], F32R)
                for kk in range(NH_LOC):   # chunked so the first wo can start early
                    nc.sync.dma_start(
                        out=wo_sb[:, kk, :],
                        in_=wo_t[kk * 128:(kk + 1) * 128, :].bitcast(F32R))

                for b in range(B):
                    for qb in range(NQB):
                        g = b * NQB + qb
                        nt = 4 * (qb + 1)            # sk tiles for this block
                        qt_tile = p2.tile([128, NH_LOC, 512], F32R, tag="qt_tile")
                        nc.sync.dma_start(
                            out=qt_tile,
                            in_=qt_dram[g].rearrange("h p t -> p h t").bitcast(F32R))
                        attn_t = p2.tile([128, NH_LOC, 4, 128], F32R, tag="attn_t")
                        for h in range(NH_LOC):
                            ps_o = ps_op.tile([128, 512], F32, tag="ps_o")
                            ps_l = ps_lp.tile([1, 512], F32, tag="ps_l")
                            for t in range(nt):
                                ps_s = ps_sp.tile([128, 512], F32, tag="ps_s")
                                nc.tensor.matmul(ps_s, kt_sb[:, b * TPB + t, :],
                                                 qt_tile[:, h, :],
                                                 start=True, stop=True)
                                v = t - 4 * qb
                                if v >= 0:   # diagonal band: causal mask
                                    nc.vector.tensor_add(ps_s, ps_s, masks_sb[:, v, :])
                                et = p2e.tile([128, 512], F32R, tag="et")
                                nc.scalar.activation(et, ps_s, AF.Exp, scale=SOFTMAX_SCALE)
                                nc.tensor.matmul(ps_o, v_sb[:, b * TPB + t, :], et,
                                                 start=(t == 0), stop=(t == nt - 1))
                                nc.tensor.matmul(ps_l, ones_sb[:, 0:1], et,
                                                 start=(t == 0), stop=(t == nt - 1))
                            lr = p2l.tile([1, 512], F32R, tag="lr")
                            nc.scalar.copy(lr, ps_l)
                            ps_b = ps_bp.tile([128, 512], F32, tag="ps_b")
                            nc.tensor.matmul(ps_b, ones_sb[0:1, :], lr, start=True, stop=True)
                            rb = p2l.tile([128, 512], F32, tag="rb")
                            nc.vector.reciprocal_approx_fast(out=rb, in_=ps_b)
                            nc.vector.tensor_mul(
                                attn_t[:, h].rearrange("p r t -> p (r t)"), ps_o, rb)
                        # wo projection for the four token tiles
                        for r in range(4):
                            tt = b * TPB + qb * 4 + r
                            o_sb = p2.tile([128, DIM], F32, tag="o_sb")
                            for n in range(DIM // 512):
                                ps_w = ps_wp.tile([128, 512], F32, tag="ps_w")
                                for kk in range(NH_LOC):
                                    nc.tensor.matmul(ps_w, attn_t[:, kk, r, :],
                                                     wo_sb[:, kk, n * 512:(n + 1) * 512],
                                                     start=(kk == 0), stop=(kk == NH_LOC - 1))
                                nc.scalar.copy(o_sb[:, n * 512:(n + 1) * 512], ps_w)
                            nc.sync.dma_start(out=out_d[tt * 128:(tt + 1) * 128, :], in_=o_sb)

    nc.compile()
    return nc


def host_prepare(x, wq, wk, wv, wo, freqs_cos, freqs_sin, B, S):
    """Build per-core in_maps. Weights nn.Linear-style [out, in]."""
    NQB = S // 512
    NTG = B * NQB
    n_heads = wq.shape[0] // HD
    n_kv = wk.shape[0] // HD
    hpc = n_heads // N_CORES       # q heads per core (4)
    kpc = n_kv // N_CORES          # kv heads per core (1)

    # deinterleave rope pairs: feature order (2i) first then (2i+1), per head
    de = np.concatenate([np.arange(0, HD, 2), np.arange(1, HD, 2)])

    xf = np.ascontiguousarray(x.reshape(B * S, DIM))
    # x^T tiled: [g, kq, k, p, t]
    x_t = np.ascontiguousarray(
        xf.T.reshape(NKQ, KPQ, 128, NTG, 512).transpose(3, 0, 1, 2, 4))

    cos = np.repeat(freqs_cos, 2, axis=1)   # [S, 128] interleaved dup
    sin = np.repeat(freqs_sin, 2, axis=1)
    cc = cos[:, de]                                             # deinterleaved
    ss = sin.copy()
    ss[:, 0::2] *= -1.0                                         # [-sin, +sin]
    ss = ss[:, de]
    cct = np.ascontiguousarray(cc.T)                            # [128, S]
    sst = np.ascontiguousarray(ss.T)

    ident = np.eye(128, dtype=np.float32)
    ones = np.ones((128, 128), dtype=np.float32)
    swap = np.zeros((128, 128), dtype=np.float32)               # half rotation
    swap[np.arange(64), np.arange(64, 128)] = 1.0
    swap[np.arange(64, 128), np.arange(64)] = 1.0
    # transposed-orientation causal masks: scores^T [sk within tile, sq in 512]
    r_idx = np.arange(128)[:, None]
    j_idx = np.arange(512)[None, :]
    masks = np.stack([
        np.where(v * 128 + r_idx <= j_idx, 0.0, -1e30).astype(np.float32)
        for v in range(4)])

    in_maps = []
    for cidx in range(N_CORES):
        qs = slice(cidx * hpc * HD, (cidx + 1) * hpc * HD)
        ks = slice(cidx * kpc * HD, (cidx + 1) * kpc * HD)
        wq_c = wq[qs].reshape(hpc, HD, DIM)[:, de, :].reshape(hpc * HD, DIM)
        wk_c = wk[ks].reshape(kpc, HD, DIM)[:, de, :].reshape(kpc * HD, DIM)
        wv_c = wv[ks]
        wkv_c = np.concatenate([wk_c, wv_c], axis=0)
        wo_c = wo[:, qs]
        in_maps.append({
            "x_t": x_t,
            "wq_t": np.ascontiguousarray(wq_c.T),
            "wkv_t": np.ascontiguousarray(wkv_c.T),
            "wo_t": np.ascontiguousarray(wo_c.T),
            "cct": cct.astype(np.float32),
            "sst": sst.astype(np.float32),
            "ident": ident,
            "ones": ones,
            "swap": swap,
            "masks": masks,
        })
    return in_maps


_CACHE = {}


def run(inputs, trace=False, trace_cores=None):
    x = np.asarray(inputs["x"], dtype=np.float32)
    B, S, _ = x.shape
    key = (B, S)
    if key not in _CACHE:
        _CACHE[key] = build_nc(B, S)
    nc = _CACHE[key]
    in_maps = host_prepare(
        x, np.asarray(inputs["wq"], np.float32), np.asarray(inputs["wk"], np.float32),
        np.asarray(inputs["wv"], np.float32), np.asarray(inputs["wo"], np.float32),
        np.asarray(inputs["freqs_cos"], np.float32),
        np.asarray(inputs["freqs_sin"], np.float32), B, S)
    res = bass_utils.run_bass_kernel_spmd(
        nc, in_maps, core_ids=list(range(N_CORES)), trace=trace,
        trace_cores=trace_cores)
    acc = np.zeros((B * S, DIM), dtype=np.float64)
    for r in res.results:
        acc += r["out"].astype(np.float64)
    out = acc.astype(np.float32).reshape(B, S, DIM)
    return out, res


def kernel(**inputs) -> np.ndarray:
    assert int(inputs.get("start_pos", 0)) == 0
    out, _ = run(inputs, trace=False)
    return out


# revision 11
# speedup vs baseline: 1.5006x; 1.0708x over previous
"""Trainium2 Bass kernel for nn_Attention_38276748542551.

Llama-style GQA attention block (DIM=4096, 32 q-heads, 8 kv-heads, hd=128,
b=2, s=2048, start_pos=0), tensor-parallel over heads across 8 NeuronCores:
each core owns 4 q-heads / 1 kv-head (wq/wk/wv output-dim shard, wo
input-dim shard) and computes a full [b*s, 4096] partial of the wo output;
the all-reduce is done on the host after gathering the 8 partials.

All matmuls run in float32r (fp32 with 11-bit mantissa; measured 1.3e-4
rel-err over K=4096 — same accuracy as the PE's plain-fp32 mode, 4x faster;
232 ns/matmul at N=512 warm).

Device dataflow per core (everything feature-major, moving dim = 512 tokens):
  phase 1 (per 512-token block): Q/K/V projections with the weight k-tile
  stationary and x^T (host pre-transposed) moving -> outputs land [feat, tok]
  -> rope applied in feature-major form: pair-swap is a 128x128 permutation
  matmul (the even/odd deinterleave is baked into the weight sharding, so the
  swap is a partition half-rotation), then two multiplies + add against
  transposed rope tables -> Q^T spilled to DRAM scratch, K^T resident in
  SBUF, V PE-transposed back to token-major and resident in SBUF.
  phase 2 (per batch, per 512-token query block, per head): scores computed
  TRANSPOSED [sk, sq] per 128-sk-tile (lhsT = K^T tile, rhs = Q^T block,
  N=512) -> causal mask add on diagonal-band tiles -> exp (ACT) -> PV
  accumulation (lhsT = V tile, N=512) and row-sum accumulation (lhsT = ones
  column) on the PE -> denominator broadcast via a K=1 ones-matmul ->
  approx-reciprocal + multiply -> attn^T, feature-major.
  phase 3 (per 128-token tile): wo projection (lhsT = attn^T, rhs = wo^T,
  8x N=512 chunks x 4 k-tiles) -> DMA partial out.
"""
import sys
import numpy as np

sys.path.insert(0, "/opt/trn_rl_repo")

import concourse.bass as bass  # noqa: E402
import concourse.tile as tile  # noqa: E402
from concourse import bacc, mybir  # noqa: E402
from concourse import bass_utils  # noqa: E402

F32 = mybir.dt.float32
F32R = mybir.dt.float32r
AF = mybir.ActivationFunctionType

DIM = 4096
NK = DIM // 128          # contraction k-tiles (32)
NKQ = 4                  # k quarters
KPQ = NK // NKQ          # k-tiles per quarter (8)
HD = 128                 # head dim
NH_LOC = 4               # q heads per core
QDIM = NH_LOC * HD       # 512
KVDIM = 2 * HD           # K and V projected together, 256
N_CORES = 8
SOFTMAX_SCALE = 1.0 / np.sqrt(HD)


def build_nc(B=2, S=2048):
    """Build the per-core Bass program (identical across cores; data differs)."""
    NT = B * S // 128            # 128-token tiles total
    TPB = S // 128               # 128-token tiles per batch
    NQB = S // 512               # 512-token blocks per batch
    NTG = B * NQB                # 512-token blocks total

    nc = bacc.Bacc("TRN2", target_bir_lowering=False, debug=False,
                   enable_asserts=False, num_devices=N_CORES)

    x_t = nc.dram_tensor("x_t", [NTG, NKQ, KPQ, 128, 512], F32, kind="ExternalInput").ap()
    wq_t = nc.dram_tensor("wq_t", [DIM, QDIM], F32, kind="ExternalInput").ap()
    wkv_t = nc.dram_tensor("wkv_t", [DIM, KVDIM], F32, kind="ExternalInput").ap()
    wo_t = nc.dram_tensor("wo_t", [QDIM, DIM], F32, kind="ExternalInput").ap()
    cct_d = nc.dram_tensor("cct", [128, S], F32, kind="ExternalInput").ap()
    sst_d = nc.dram_tensor("sst", [128, S], F32, kind="ExternalInput").ap()
    ident_d = nc.dram_tensor("ident", [128, 128], F32, kind="ExternalInput").ap()
    ones_d = nc.dram_tensor("ones", [128, 128], F32, kind="ExternalInput").ap()
    swap_d = nc.dram_tensor("swap", [128, 128], F32, kind="ExternalInput").ap()
    masks_d = nc.dram_tensor("masks", [4, 128, 512], F32, kind="ExternalInput").ap()
    out_d = nc.dram_tensor("out", [B * S, DIM], F32, kind="ExternalOutput").ap()

    with tile.TileContext(nc) as tc:
        with tc.tile_pool(name="singles", bufs=1) as singles, \
             tc.tile_pool(name="dram", bufs=1, space="DRAM") as dpool:
            ident = singles.tile([128, 128], F32R)
            nc.sync.dma_start(out=ident, in_=ident_d.bitcast(F32R))
            ones_sb = singles.tile([128, 128], F32R)
            nc.sync.dma_start(out=ones_sb, in_=ones_d.bitcast(F32R))
            swap_sb = singles.tile([128, 128], F32R)
            nc.sync.dma_start(out=swap_sb, in_=swap_d.bitcast(F32R))
            kt_sb = singles.tile([128, NT, 128], F32R)   # K^T: [hd, tile, tok]
            v_sb = singles.tile([128, NT, 128], F32R)    # V: [tok, tile, hd]
            qt_dram = dpool.tile([NTG, NH_LOC, 128, 512], F32)

            # ---------------- phase 1: projections + rope (feature-major) ----------------
            with tc.tile_pool(name="p1w", bufs=1) as p1w, \
                 tc.tile_pool(name="p1", bufs=3) as p1, \
                 tc.tile_pool(name="p1r", bufs=2) as p1r, \
                 tc.tile_pool(name="p1t", bufs=2) as p1t, \
                 tc.tile_pool(name="ps_acc", bufs=6, space="PSUM") as ps_accp, \
                 tc.tile_pool(name="ps_misc", bufs=2, space="PSUM") as ps_miscp:

                def load_xs(g, kq):
                    t_ = p1.tile([128, KPQ, 512], F32R, tag="xs")
                    nc.sync.dma_start(
                        out=t_, in_=x_t[g, kq].rearrange("k p t -> p k t").bitcast(F32R))
                    return t_

                # interleave first xs quarters with the weight chunk loads so
                # neither starves the other on the DMA queue
                wq_sb = p1w.tile([128, NK, QDIM], F32R)
                wkv_sb = p1w.tile([128, NK, KVDIM], F32R)
                xs_pre = [load_xs(0, 0)]

                def load_wq(qc):
                    ksl = slice(qc * (NK // 4) * 128, (qc + 1) * (NK // 4) * 128)
                    nc.sync.dma_start(
                        out=wq_sb[:, qc * (NK // 4):(qc + 1) * (NK // 4), :],
                        in_=wq_t[ksl, :].rearrange("(k p) n -> p k n", p=128).bitcast(F32R))

                def load_wkv(hc):
                    ksl = slice(hc * (NK // 2) * 128, (hc + 1) * (NK // 2) * 128)
                    nc.sync.dma_start(
                        out=wkv_sb[:, hc * (NK // 2):(hc + 1) * (NK // 2), :],
                        in_=wkv_t[ksl, :].rearrange("(k p) n -> p k n", p=128).bitcast(F32R))

                load_wq(0)
                xs_pre.append(load_xs(0, 1))
                load_wkv(0)
                load_wq(1)
                xs_pre.append(load_xs(0, 2))
                load_wkv(1)
                load_wq(2)
                load_wq(3)

                nload = 3   # next (g*NKQ+kq) index to load; keep 2 in flight
                for g in range(NTG):
                    # per-block rope table slices [128, 512]
                    pos = (g % NQB) * 512
                    cct = p1t.tile([128, 512], F32, tag="cct")
                    nc.sync.dma_start(out=cct, in_=cct_d[:, pos:pos + 512])
                    sst = p1t.tile([128, 512], F32, tag="sst")
                    nc.sync.dma_start(out=sst, in_=sst_d[:, pos:pos + 512])

                    acc = [ps_accp.tile([128, 512], F32, tag="acc", name=f"acc{g}_{j}") for j in range(6)]
                    for kq in range(NKQ):
                        xs = xs_pre.pop(0)
                        if nload < NTG * NKQ:
                            xs_pre.append(load_xs(nload // NKQ, nload % NKQ))
                            nload += 1
                        for k in range(KPQ):
                            kt = kq * KPQ + k
                            st = (kt == 0)
                            sp = (kt == NK - 1)
                            for h in range(NH_LOC):
                                nc.tensor.matmul(acc[h], wq_sb[:, kt, h * 128:(h + 1) * 128],
                                                 xs[:, k, :], start=st, stop=sp)
                            nc.tensor.matmul(acc[4], wkv_sb[:, kt, 0:128],
                                             xs[:, k, :], start=st, stop=sp)
                            nc.tensor.matmul(acc[5], wkv_sb[:, kt, 128:256],
                                             xs[:, k, :], start=st, stop=sp)

                    # rope Q (4 heads) + K in feature-major; V transpose to token-major
                    for j in range(5):   # 0..3 = q heads, 4 = K
                        f = p1r.tile([128, 512], F32R, tag="f")
                        nc.scalar.copy(f, acc[j])
                        ps_sw = ps_miscp.tile([128, 512], F32, tag="misc")
                        nc.tensor.matmul(ps_sw, swap_sb, f, start=True, stop=True)
                        t1 = p1r.tile([128, 512], F32, tag="t1")
                        nc.vector.tensor_mul(t1, f.bitcast(F32), cct)
                        t2 = p1r.tile([128, 512], F32, tag="t2")
                        nc.vector.tensor_mul(t2, ps_sw, sst)
                        if j < NH_LOC:
                            qr = p1r.tile([128, 512], F32R, tag="qr")
                            nc.vector.tensor_add(qr, t1, t2)
                            nc.sync.dma_start(out=qt_dram[g, j].bitcast(F32R), in_=qr)
                        else:
                            nc.vector.tensor_add(
                                kt_sb[:, 4 * g:4 * g + 4, :].rearrange("p a t -> p (a t)"),
                                t1, t2)
                    vf = p1r.tile([128, 512], F32R, tag="f")
                    nc.scalar.copy(vf, acc[5])
                    for r in range(4):
                        ps_vt = ps_miscp.tile([128, 512], F32R, tag="misc")
                        nc.tensor.transpose(ps_vt[:, 0:128], vf[:, r * 128:(r + 1) * 128], ident)
                        nc.scalar.copy(v_sb[:, 4 * g + r, :], ps_vt[:, 0:128])

            # ------------- phase 2/3: attention (transposed scores) + wo -------------
            with tc.tile_pool(name="p2w", bufs=1) as p2w, \
                 tc.tile_pool(name="p2", bufs=2) as p2, \
                 tc.tile_pool(name="p2e", bufs=3) as p2e, \
                 tc.tile_pool(name="p2l", bufs=3) as p2l, \
                 tc.tile_pool(name="ps_s", bufs=3, space="PSUM") as ps_sp, \
                 tc.tile_pool(name="ps_o", bufs=2, space="PSUM") as ps_op, \
                 tc.tile_pool(name="ps_l", bufs=1, space="PSUM") as ps_lp, \
                 tc.tile_pool(name="ps_w", bufs=2, space="PSUM") as ps_wp:
                masks_sb = p2w.tile([128, 4, 512], F32)
                nc.sync.dma_start(out=masks_sb, in_=masks_d.rearrange("v p n -> p v n"))
                wo_sb = p2w.tile([128, NH_LOC, DIM], F32R)
                for kk in range(NH_LOC):   # chunked so the first wo can start early
                    nc.sync.dma_start(
                        out=wo_sb[:, kk, :],
                        in_=wo_t[kk * 128:(kk + 1) * 128, :].bitcast(F32R))

                for b in range(B):
                    for qb in range(NQB):
                        g = b * NQB + qb
                        nt = 4 * (qb + 1)            # sk tiles for this block
                        qt_tile = p2.tile([128, NH_LOC, 512], F32R, tag="qt_tile")
                        nc.sync.dma_start(
                            out=qt_tile,
                            in_=qt_dram[g].rearrange("h p t -> p h t").bitcast(F32R))
                        attn_t = p2.tile([128, NH_LOC, 4, 128], F32R, tag="attn_t")
                        for h in range(NH_LOC):
                            ps_o = ps_op.tile([128, 512], F32, tag="ps_o")
                            ps_l = ps_lp.tile([1, 512], F32, tag="ps_l")
                            for t in range(nt):
                                ps_s = ps_sp.tile([128, 512], F32, tag="ps_s")
                                nc.tensor.matmul(ps_s, kt_sb[:, b * TPB + t, :],
                                                 qt_tile[:, h, :],
                                                 start=True, stop=True)
                                v = t - 4 * qb
                                if v >= 0:   # diagonal band: causal mask
                                    nc.vector.tensor_add(ps_s, ps_s, masks_sb[:, v, :])
                                et = p2e.tile([128, 512], F32R, tag="et")
                                nc.scalar.activation(et, ps_s, AF.Exp, scale=SOFTMAX_SCALE)
                                nc.tensor.matmul(ps_o, v_sb[:, b * TPB + t, :], et,
                                                 start=(t == 0), stop=(t == nt - 1))
                                nc.tensor.matmul(ps_l, ones_sb[:, 0:1], et,
                                                 start=(t == 0), stop=(t == nt - 1))
                            lr = p2l.tile([1, 512], F32R, tag="lr")
                            nc.scalar.copy(lr, ps_l)
                            ps_b = ps_wp.tile([128, 512], F32, tag="ps_w", name=f"ps_b{g}_{h}")
                            nc.tensor.matmul(ps_b, ones_sb[0:1, :], lr, start=True, stop=True)
                            rb = p2l.tile([128, 512], F32, tag="rb")
                            nc.vector.reciprocal_approx_fast(out=rb, in_=ps_b)
                            nc.vector.tensor_mul(
                                attn_t[:, h].rearrange("p r t -> p (r t)"), ps_o, rb)
                        # wo projection for the four token tiles
                        for r in range(4):
                            tt = b * TPB + qb * 4 + r
                            o_sb = p2.tile([128, DIM], F32, tag="o_sb")
                            for n in range(DIM // 512):
                                ps_w = ps_wp.tile([128, 512], F32, tag="ps_w")
                                for kk in range(NH_LOC):
                                    nc.tensor.matmul(ps_w, attn_t[:, kk, r, :],
                                                     wo_sb[:, kk, n * 512:(n + 1) * 512],
                                                     start=(kk == 0), stop=(kk == NH_LOC - 1))
                                nc.scalar.copy(o_sb[:, n * 512:(n + 1) * 512], ps_w)
                            nc.sync.dma_start(out=out_d[tt * 128:(tt + 1) * 128, :], in_=o_sb)

    nc.compile()
    return nc


def host_prepare(x, wq, wk, wv, wo, freqs_cos, freqs_sin, B, S):
    """Build per-core in_maps. Weights nn.Linear-style [out, in]."""
    NQB = S // 512
    NTG = B * NQB
    n_heads = wq.shape[0] // HD
    n_kv = wk.shape[0] // HD
    hpc = n_heads // N_CORES       # q heads per core (4)
    kpc = n_kv // N_CORES          # kv heads per core (1)

    # deinterleave rope pairs: feature order (2i) first then (2i+1), per head
    de = np.concatenate([np.arange(0, HD, 2), np.arange(1, HD, 2)])

    xf = np.ascontiguousarray(x.reshape(B * S, DIM))
    # x^T tiled: [g, kq, k, p, t]
    x_t = np.ascontiguousarray(
        xf.T.reshape(NKQ, KPQ, 128, NTG, 512).transpose(3, 0, 1, 2, 4))

    cos = np.repeat(freqs_cos, 2, axis=1)   # [S, 128] interleaved dup
    sin = np.repeat(freqs_sin, 2, axis=1)
    cc = cos[:, de]                                             # deinterleaved
    ss = sin.copy()
    ss[:, 0::2] *= -1.0                                         # [-sin, +sin]
    ss = ss[:, de]
    cct = np.ascontiguousarray(cc.T)                            # [128, S]
    sst = np.ascontiguousarray(ss.T)

    ident = np.eye(128, dtype=np.float32)
    ones = np.ones((128, 128), dtype=np.float32)
    swap = np.zeros((128, 128), dtype=np.float32)               # half rotation
    swap[np.arange(64), np.arange(64, 128)] = 1.0
    swap[np.arange(64, 128), np.arange(64)] = 1.0
    # transposed-orientation causal masks: scores^T [sk within tile, sq in 512]
    r_idx = np.arange(128)[:, None]
    j_idx = np.arange(512)[None, :]
    masks = np.stack([
        np.where(v * 128 + r_idx <= j_idx, 0.0, -1e30).astype(np.float32)
        for v in range(4)])

    in_maps = []
    for cidx in range(N_CORES):
        qs = slice(cidx * hpc * HD, (cidx + 1) * hpc * HD)
        ks = slice(cidx * kpc * HD, (cidx + 1) * kpc * HD)
        wq_c = wq[qs].reshape(hpc, HD, DIM)[:, de, :].reshape(hpc * HD, DIM)
        wk_c = wk[ks].reshape(kpc, HD, DIM)[:, de, :].reshape(kpc * HD, DIM)
        wv_c = wv[ks]
        wkv_c = np.concatenate([wk_c, wv_c], axis=0)
        wo_c = wo[:, qs]
        in_maps.append({
            "x_t": x_t,
            "wq_t": np.ascontiguousarray(wq_c.T),
            "wkv_t": np.ascontiguousarray(wkv_c.T),
            "wo_t": np.ascontiguousarray(wo_c.T),
            "cct": cct.astype(np.float32),
            "sst": sst.astype(np.float32),
            "ident": ident,
            "ones": ones,
            "swap": swap,
            "masks": masks,
        })
    return in_maps


_CACHE = {}


def run(inputs, trace=False, trace_cores=None):
    x = np.asarray(inputs["x"], dtype=np.float32)
    B, S, _ = x.shape
    key = (B, S)
    if key not in _CACHE:
        _CACHE[key] = build_nc(B, S)
    nc = _CACHE[key]
    in_maps = host_prepare(
        x, np.asarray(inputs["wq"], np.float32), np.asarray(inputs["wk"], np.float32),
        np.asarray(inputs["wv"], np.float32), np.asarray(inputs["wo"], np.float32),
        np.asarray(inputs["freqs_cos"], np.float32),
        np.asarray(inputs["freqs_sin"], np.float32), B, S)
    res = bass_utils.run_bass_kernel_spmd(
        nc, in_maps, core_ids=list(range(N_CORES)), trace=trace,
        trace_cores=trace_cores)
    acc = np.zeros((B * S, DIM), dtype=np.float64)
    for r in res.results:
        acc += r["out"].astype(np.float64)
    out = acc.astype(np.float32).reshape(B, S, DIM)
    return out, res


def kernel(**inputs) -> np.ndarray:
    assert int(inputs.get("start_pos", 0)) == 0
    out, _ = run(inputs, trace=False)
    return out
